# revision 1
# baseline (speedup 1.0000x reference)
"""Trainium2 Bass kernel for a masked-attention block (MAB).

Computation (per batch element, all fp32):
    Q = X@Wq + bq ; K = Y@Wk + bk ; V = Y@Wv + bv
    logits = per-head Qh@Kh^T / 32, masked keys -> -inf, softmax over keys
    attn   = A @ Vh (concat heads)
    O1 = LN(Q + attn; g1,b1)
    O  = LN(O1 + relu(O1@Wo + bo); g2,b2)

Sharding: pure data-parallel, one batch element per NeuronCore (B=8 = 8 cores).

On-device dataflow is "feature-major": activations live in SBUF transposed
([model_dim -> 8x128 partitions, token -> free]).  With weights in natural
layout every matmul chains without any transposes:
    actT_out[n, t] = sum_d W[d, n] * actT_in[d, t]   (lhsT=W, rhs=actT_in)
Attention also chains: logitsT[k, q] from (lhsT=KT_h, rhs=QT_h) single
128-contraction; exp on ACT (mask folded in as a per-partition bias);
AV from (lhsT=V_natural, rhs=expT).  The softmax denominator and the
LayerNorm stats are partition-dim reductions done with all-ones stationary
matmuls (which also broadcast the result across partitions for free).
All matmuls use float32r (FP22 truncation) which runs at full PE rate for
moving free-dim >= 256.

The host transposes X/Y on the way in and the output on the way out, and
converts the bool mask into an additive f32 bias (0 / -1e4).
"""

import math
import numpy as np
from contextlib import ExitStack

import concourse.bass as bass
import concourse.mybir as mybir
import concourse.tile as tile
from concourse import bacc
from concourse.bass_utils import run_bass_kernel_spmd

P = 128
NX = 1024
NY = 1024
DIM = 1024
H = 8
KO = DIM // P          # 8 partition sub-tiles of the model dim
QC = 512               # moving-operand chunk (fp32 max free dim)
NQC = NX // QC         # 2
F32 = mybir.dt.float32
F32R = mybir.dt.float32r
BF16 = mybir.dt.bfloat16
# ldw-opt dedupes adjacent same-stationary LDWEIGHTS, but it is disabled in
# every production compile config here and we could not A/B-verify it on
# hardware before the time budget ran out — keep it off.
ENABLE_LDW_OPT = False
AF = mybir.ActivationFunctionType
ALU = mybir.AluOpType
SCALE = 1.0 / 32.0     # 1/sqrt(DIM)
EPS = 1e-5


def _r(ap):
    return ap.bitcast(F32R)


_LDW_PATCHED = False


def _patch_ldw_opt():
    """walrus ships with --enable-ldw-opt=false hardcoded; with our loop
    order same-stationary matmuls are adjacent, so deduping LDWEIGHTS is a
    large PE win.  Rewrite the flag on the walrus command line."""
    global _LDW_PATCHED
    if _LDW_PATCHED or not ENABLE_LDW_OPT:
        return
    import concourse.bass_utils as _bu
    _orig = _bu.run_command

    def _run_command(argv, **kwargs):
        argv = ["--enable-ldw-opt=true" if a == "--enable-ldw-opt=false" else a
                for a in argv]
        return _orig(argv, **kwargs)

    _bu.run_command = _run_command
    _LDW_PATCHED = True


def _build():
    _patch_ldw_opt()
    nc = bacc.Bacc("TRN2", target_bir_lowering=False, debug=False,
                   enable_asserts=False)

    # ---- DRAM I/O (per-core shapes) ----
    XT = nc.dram_tensor("XT", [DIM, NX], F32, kind="ExternalInput").ap()
    YT = nc.dram_tensor("YT", [DIM, NY], F32, kind="ExternalInput").ap()
    MB = nc.dram_tensor("MB", [NY], F32, kind="ExternalInput").ap()
    Wd = {}
    for w in ("Wq", "Wk", "Wv", "Wo"):
        Wd[w] = nc.dram_tensor(w, [DIM, DIM], F32, kind="ExternalInput").ap()
    Vecs = {}
    for vname in ("bq", "bk", "bv", "bo", "g1", "b1", "g2", "b2"):
        Vecs[vname] = nc.dram_tensor(vname, [DIM], F32, kind="ExternalInput").ap()
    OT = nc.dram_tensor("OT", [DIM, NX], F32, kind="ExternalOutput").ap()

    xt3 = XT.rearrange("(ko p) q -> p ko q", p=P)
    yt3 = YT.rearrange("(ko p) q -> p ko q", p=P)
    wq3 = Wd["Wq"].rearrange("(ko p) d -> p ko d", p=P)
    wk3 = Wd["Wk"].rearrange("(ko p) d -> p ko d", p=P)
    wv3 = Wd["Wv"].rearrange("(ko p) d -> p ko d", p=P)
    wo3 = Wd["Wo"].rearrange("(ko p) d -> p ko d", p=P)
    ot3 = OT.rearrange("(do p) q -> p do q", p=P)

    with tile.TileContext(nc) as tc:
        with ExitStack() as octx:
            const = octx.enter_context(tc.tile_pool(name="const", bufs=1))
            actp = octx.enter_context(tc.tile_pool(name="act", bufs=3))

            # ---- constants ----
            # walrus requires every writer of an fp32r-matmul operand to have
            # an fp32r-tagged output AP; memset can't write f32r, so round
            # the ones through a copy
            ones128 = const.tile([P, P], F32, tag="ones", name="ones128")
            ones_tmp = const.tile([P, P], F32, tag="onest", name="ones_tmp")
            nc.vector.memset(ones_tmp, 1.0)
            nc.vector.tensor_copy(_r(ones128), ones_tmp)
            ones_bf = const.tile([P, P], BF16, tag="onesbf", name="ones_bf")
            nc.vector.memset(ones_bf, 1.0)
            eps_sb = const.tile([P, 1], F32, tag="eps", name="eps_sb")
            nc.vector.memset(eps_sb, EPS)

            def vec_pko(name):
                t = const.tile([P, KO], F32, tag=f"v_{name}", name=f"{name}_sb")
                nc.sync.dma_start(t, Vecs[name].rearrange("(ko p) -> p ko", p=P))
                return t

            mb_sb = const.tile([P, KO], F32, tag="v_mb", name="mb_sb")
            nc.sync.dma_start(mb_sb, MB.rearrange("(ko p) -> p ko", p=P))
            bq_sb = vec_pko("bq")
            bk_sb = vec_pko("bk")
            bo_sb = vec_pko("bo")
            g1_sb = vec_pko("g1")
            b1_sb = vec_pko("b1")
            g2_sb = vec_pko("g2")
            b2_sb = vec_pko("b2")
            bv_sb = const.tile([1, DIM], F32, tag="v_bv", name="bv_sb")
            nc.sync.dma_start(_r(bv_sb),
                              _r(Vecs["bv"].rearrange("(one n) -> one n", one=1)))

            # ---- big feature-major activation tiles (rotating slots) ----
            qt = actp.tile([P, KO, NX], F32, tag="big", name="qt")
            ktm = actp.tile([P, KO, NY], F32, tag="big", name="ktm")
            vm = actp.tile([P, KO, DIM], BF16, tag="big", name="vm")

            # ================= Phase 1: Q, K, V projections =================
            with tc.tile_pool(name="io", bufs=1) as iop, \
                 tc.tile_pool(name="w1", bufs=2) as wp, \
                 tc.tile_pool(name="gp1", bufs=8, space="PSUM") as pp:
                xt = iop.tile([P, KO, NX], F32, tag="xt", name="xt")
                yt = iop.tile([P, KO, NY], F32, tag="yt", name="yt")
                for k in range(KO):
                    nc.sync.dma_start(_r(xt[:, k, :]), _r(xt3[:, k, :]))
                for k in range(KO):
                    nc.sync.dma_start(_r(yt[:, k, :]), _r(yt3[:, k, :]))

                def proj_featmajor(w3, rhs_sb, out_sb, bias_sb, label):
                    # out_sb[p, do, q] (+= bias[do*128+p]) = sum_k W[k, d] rhs[k, q]
                    # qc innermost: both uses of each stationary tile are
                    # back-to-back so ldw-opt can dedupe the LDWEIGHTS
                    for dg in range(2):
                        wt = wp.tile([P, KO, QC], F32, tag="w", name=f"w_{label}{dg}")
                        for k in range(KO):
                            nc.sync.dma_start(_r(wt[:, k, :]),
                                              _r(w3[:, k, dg * QC:(dg + 1) * QC]))
                        for d4 in range(4):
                            pss = [pp.tile([P, QC], F32, tag="ps",
                                           name=f"ps_{label}{dg}{d4}{qc}")
                                   for qc in range(NQC)]
                            for k in range(KO):
                                for qc in range(NQC):
                                    qs = slice(qc * QC, (qc + 1) * QC)
                                    nc.tensor.matmul(
                                        pss[qc],
                                        lhsT=_r(wt[:, k, d4 * P:(d4 + 1) * P]),
                                        rhs=_r(rhs_sb[:, k, qs]),
                                        start=(k == 0), stop=(k == KO - 1))
                            do = dg * 4 + d4
                            for qc in range(NQC):
                                qs = slice(qc * QC, (qc + 1) * QC)
                                nc.scalar.activation(
                                    _r(out_sb[:, do, qs]), pss[qc], AF.Identity,
                                    bias=bias_sb[:, do:do + 1], scale=1.0)

                proj_featmajor(wq3, xt, qt, bq_sb, "q")
                proj_featmajor(wk3, yt, ktm, bk_sb, "k")

                # V in natural (token-major) layout: V[y, n] = sum_k Y[y,k] Wv[k,n]
                # (bf16 output — only consumed by the AV matmul).  ng innermost
                # so each yt stationary tile is used twice back-to-back.
                wts = []
                for ng in range(2):
                    wt = wp.tile([P, KO, QC], F32, tag="w", name=f"w_v{ng}")
                    for k in range(KO):
                        nc.sync.dma_start(_r(wt[:, k, :]),
                                          _r(wv3[:, k, ng * QC:(ng + 1) * QC]))
                    wts.append(wt)
                for yo in range(KO):
                    pss = [pp.tile([P, QC], F32, tag="ps", name=f"ps_v{yo}{ng}")
                           for ng in range(2)]
                    for k in range(KO):
                        for ng in range(2):
                            nc.tensor.matmul(
                                pss[ng],
                                lhsT=_r(yt[:, k, yo * P:(yo + 1) * P]),
                                rhs=_r(wts[ng][:, k, :]),
                                start=(k == 0), stop=False)
                    for ng in range(2):
                        ns = slice(ng * QC, (ng + 1) * QC)
                        # fold per-free-dim bias bv with a K=1 ones matmul
                        nc.tensor.matmul(
                            pss[ng], lhsT=_r(ones128[0:1, :]), rhs=_r(bv_sb[:, ns]),
                            start=False, stop=True)
                        nc.scalar.copy(vm[:, yo, ns], pss[ng])

            # ================= Phase 2: attention =================
            with tc.tile_pool(name="zp", bufs=1) as zp:
                zt = zp.tile([P, KO, NX], F32, tag="z", name="zt")

                with tc.tile_pool(name="exp", bufs=20) as ep, \
                     tc.tile_pool(name="rcp", bufs=2) as rp, \
                     tc.tile_pool(name="lgp", bufs=2, space="PSUM") as lgp, \
                     tc.tile_pool(name="avp", bufs=1, space="PSUM") as avp, \
                     tc.tile_pool(name="rlp", bufs=1, space="PSUM") as rlp:

                    def logits_exp(h):
                        # logitsT[k, q] = sum_d KT_h[d, k] QT_h[d, q]; exp with
                        # mask bias per key (partition) and 1/32 scale.  The
                        # logits psum tile spans 2 banks so one ACT op covers
                        # the whole [128, 1024] key-slice.  exp output is bf16
                        # (feeds only the bf16 AV/denominator matmuls).
                        et = [ep.tile([P, NY], BF16, tag="exp", name=f"et{h}_{k}")
                              for k in range(KO)]
                        for kt in range(KO):
                            pl = lgp.tile([P, NX], F32, tag="lg",
                                          name=f"pl{h}{kt}")
                            for qc in range(NQC):
                                qs = slice(qc * QC, (qc + 1) * QC)
                                nc.tensor.matmul(
                                    pl[:, qs],
                                    lhsT=_r(ktm[:, h, kt * P:(kt + 1) * P]),
                                    rhs=_r(qt[:, h, qs]),
                                    start=True, stop=True)
                            nc.scalar.activation(
                                et[kt], pl, AF.Exp,
                                bias=mb_sb[:, kt:kt + 1], scale=SCALE)
                        return et

                    def denom_av(h, et):
                        # softmax denominator: accumulate the all-ones matmul
                        # over the 8 key sub-tiles -> partition-reduction AND
                        # broadcast in one shot (also keeps PE warm here)
                        pr = rlp.tile([P, NX], F32, tag="rl", name=f"pr{h}")
                        for kt in range(KO):
                            for qc in range(NQC):
                                qs = slice(qc * QC, (qc + 1) * QC)
                                nc.tensor.matmul(
                                    pr[:, qs], lhsT=ones_bf,
                                    rhs=et[kt][:, qs],
                                    start=(kt == 0), stop=(kt == KO - 1))
                        rc = rp.tile([P, NX], F32, tag="rc", name=f"rc{h}")
                        nc.vector.reciprocal_approx_fast(rc, pr)
                        # attnT_h[d, q] = sum_k V[k, d_h] expT[k, q]; then
                        # normalize by the softmax denom and add the Q residual
                        pa = avp.tile([P, NX], F32, tag="av", name=f"pa{h}")
                        for kt in range(KO):
                            for qc in range(NQC):
                                qs = slice(qc * QC, (qc + 1) * QC)
                                nc.tensor.matmul(
                                    pa[:, qs],
                                    lhsT=vm[:, kt, h * P:(h + 1) * P],
                                    rhs=et[kt][:, qs],
                                    start=(kt == 0), stop=(kt == KO - 1))
                        nc.vector.tensor_mul(_r(zt[:, h, :]), pa, rc)
                        nc.vector.tensor_add(_r(zt[:, h, :]), zt[:, h, :],
                                             qt[:, h, :])

                    # software pipeline: head h's logits/exp (PE+ACT) run while
                    # head h-1's denominator+AV (PE) wait on h-1's exp -> PE
                    # never idles long enough for HAM to re-throttle
                    prev = None
                    for h in range(H):
                        et = logits_exp(h)
                        if prev is not None:
                            denom_av(h - 1, prev)
                        prev = et
                    denom_av(H - 1, prev)

                # ---- LayerNorm over the model dim (partition direction) ----
                def layernorm(in_sb, sqp, stp, spp, emit_out):
                    for qc in range(NQC):
                        qs = slice(qc * QC, (qc + 1) * QC)
                        pmu = spp.tile([P, QC], F32, tag="pmu", name=f"pmu{qc}")
                        ps2 = spp.tile([P, QC], F32, tag="ps2", name=f"ps2{qc}")
                        for do in range(KO):
                            nc.tensor.matmul(pmu, lhsT=_r(ones128),
                                             rhs=_r(in_sb[:, do, qs]),
                                             start=(do == 0), stop=(do == KO - 1))
                        for do in range(KO):
                            sq = sqp.tile([P, QC], F32, tag="sq", name=f"sq{qc}{do}")
                            nc.vector.tensor_mul(_r(sq), in_sb[:, do, qs],
                                                 in_sb[:, do, qs])
                            nc.tensor.matmul(ps2, lhsT=_r(ones128), rhs=_r(sq),
                                             start=(do == 0), stop=(do == KO - 1))
                        mu = stp.tile([P, QC], F32, tag="mu", name=f"mu{qc}")
                        nc.vector.tensor_scalar_mul(mu, pmu, 1.0 / DIM)
                        msq = stp.tile([P, QC], F32, tag="msq", name=f"msq{qc}")
                        nc.vector.tensor_mul(msq, mu, mu)
                        sd = stp.tile([P, QC], F32, tag="sd", name=f"sd{qc}")
                        nc.vector.scalar_tensor_tensor(
                            sd, ps2, 1.0 / DIM, msq,
                            op0=ALU.mult, op1=ALU.subtract)
                        nc.scalar.activation(sd, sd, AF.Sqrt, bias=eps_sb, scale=1.0)
                        rsig = stp.tile([P, QC], F32, tag="rsig", name=f"rsig{qc}")
                        nc.vector.reciprocal_approx_fast(rsig, sd)
                        mrs = stp.tile([P, QC], F32, tag="mrs", name=f"mrs{qc}")
                        nc.vector.tensor_mul(mrs, mu, rsig)
                        for do in range(KO):
                            t = sqp.tile([P, QC], F32, tag="t", name=f"t{qc}{do}")
                            nc.vector.tensor_mul(t, in_sb[:, do, qs], rsig)
                            nc.vector.tensor_sub(t, t, mrs)
                            emit_out(do, qs, t)

                # LN1 -> o1t (feature-major)
                with tc.tile_pool(name="sq1", bufs=3) as sqp1, \
                     tc.tile_pool(name="st1", bufs=2) as stp1, \
                     tc.tile_pool(name="sp1", bufs=2, space="PSUM") as spp1:
                    o1t = actp.tile([P, KO, NX], F32, tag="big", name="o1t")

                    def emit_o1(do, qs, t):
                        nc.vector.tensor_scalar(
                            _r(o1t[:, do, qs]), t,
                            scalar1=g1_sb[:, do:do + 1],
                            scalar2=b1_sb[:, do:do + 1],
                            op0=ALU.mult, op1=ALU.add)

                    layernorm(zt, sqp1, stp1, spp1, emit_o1)

            # ================= Phase 3: output proj + LN2 =================
            with tc.tile_pool(name="w3", bufs=2) as wp3, \
                 tc.tile_pool(name="sq2", bufs=4) as sqp2, \
                 tc.tile_pool(name="st2", bufs=2) as stp2, \
                 tc.tile_pool(name="out", bufs=4) as outp, \
                 tc.tile_pool(name="gp3", bufs=4, space="PSUM") as pp3, \
                 tc.tile_pool(name="sp2", bufs=2, space="PSUM") as spp2:
                z2t = actp.tile([P, KO, NX], F32, tag="big", name="z2t")
                # HT[n, q] = sum_d Wo[d, n] O1T[d, q];  z2 = o1 + relu(H + bo)
                for ng in range(2):
                    wt = wp3.tile([P, KO, QC], F32, tag="w", name=f"w_o{ng}")
                    for k in range(KO):
                        nc.sync.dma_start(_r(wt[:, k, :]),
                                          _r(wo3[:, k, ng * QC:(ng + 1) * QC]))
                    for qc in range(NQC):
                        qs = slice(qc * QC, (qc + 1) * QC)
                        for n4 in range(4):
                            ps = pp3.tile([P, QC], F32, tag="ps",
                                          name=f"ps_o{ng}{qc}{n4}")
                            for k in range(KO):
                                nc.tensor.matmul(
                                    ps,
                                    lhsT=_r(wt[:, k, n4 * P:(n4 + 1) * P]),
                                    rhs=_r(o1t[:, k, qs]),
                                    start=(k == 0), stop=(k == KO - 1))
                            no = ng * 4 + n4
                            ht = sqp2.tile([P, QC], F32, tag="ht",
                                           name=f"ht{ng}{qc}{n4}")
                            nc.scalar.activation(ht, ps, AF.Relu,
                                                 bias=bo_sb[:, no:no + 1], scale=1.0)
                            nc.vector.tensor_add(_r(z2t[:, no, qs]), ht,
                                                 o1t[:, no, qs])

                def emit_o2(do, qs, t):
                    o = outp.tile([P, QC], F32, tag="o", name=f"o{do}")
                    nc.vector.tensor_scalar(
                        o, t,
                        scalar1=g2_sb[:, do:do + 1],
                        scalar2=b2_sb[:, do:do + 1],
                        op0=ALU.mult, op1=ALU.add)
                    nc.sync.dma_start(ot3[:, do, qs], o)

                layernorm(z2t, sqp2, stp2, spp2, emit_o2)

    nc.compile()
    return nc


_CACHE = {}


def _get_nc():
    if "nc" not in _CACHE:
        _CACHE["nc"] = _build()
    return _CACHE["nc"]


def make_in_maps(X, Y, mask, Wq, bq, Wk, bk, Wv, bv, Wo, bo, g1, b1, g2, b2):
    f = lambda a: np.ascontiguousarray(np.asarray(a, dtype=np.float32))
    shared = {
        "Wq": f(Wq), "Wk": f(Wk), "Wv": f(Wv), "Wo": f(Wo),
        "bq": f(bq), "bk": f(bk), "bv": f(bv), "bo": f(bo),
        "g1": f(g1), "b1": f(b1), "g2": f(g2), "b2": f(b2),
    }
    X = np.asarray(X, dtype=np.float32)
    Y = np.asarray(Y, dtype=np.float32)
    mask = np.asarray(mask)
    in_maps = []
    for b in range(8):
        mb = np.where(mask[b], np.float32(-1e4), np.float32(0.0)).astype(np.float32)
        in_maps.append({
            "XT": np.ascontiguousarray(X[b].T),
            "YT": np.ascontiguousarray(Y[b].T),
            "MB": mb,
            **shared,
        })
    return in_maps


def kernel(X, Y, mask, Wq, bq, Wk, bk, Wv, bv, Wo, bo, g1, b1, g2, b2,
           _trace=False):
    nc = _get_nc()
    in_maps = make_in_maps(X, Y, mask, Wq, bq, Wk, bk, Wv, bv, Wo, bo,
                           g1, b1, g2, b2)
    res = run_bass_kernel_spmd(nc, in_maps, core_ids=list(range(8)),
                               trace=_trace)
    out = np.stack([np.ascontiguousarray(res.results[b]["OT"].T)
                    for b in range(8)]).astype(np.float32)
    if _trace:
        return out, res
    return out



# revision 14
# speedup vs baseline: 1.4610x; 1.4610x over previous
"""Trainium2 Bass kernel for a masked-attention block (MAB).

Computation (per batch element):
    Q = X@Wq + bq ; K = Y@Wk + bk ; V = Y@Wv + bv
    logits = per-head Qh@Kh^T / 32, masked keys -> -inf, softmax over keys
    attn   = A @ Vh (concat heads)
    O1 = LN(Q + attn; g1,b1)
    O  = LN(O1 + relu(O1@Wo + bo); g2,b2)

Sharding: pure data-parallel, one batch element per NeuronCore (B=8 = 8 cores).

On-device dataflow is "feature-major": activations live in SBUF transposed
([model_dim -> 8x128 partitions, token -> free]) so every matmul chains
without transposes.  Softmax denominators and LayerNorm stats are
partition-dim reductions done with all-ones stationary matmuls (which also
broadcast the result across partitions for free).

v2 changes vs the f32 baseline:
  * bf16 activations/weights end to end (same PE rate, 2x DVE rate, half
    the DMA bytes).  Host converts on the way in; output stays f32.
  * fp8(e4m3) + DoubleRow for the K/V projections, the AV matmul and the
    softmax denominator (2 contraction-tiles per matmul -> ~2x PE).  Wk/Wv
    are pre-scaled by 32 on the host so their values sit in e4m3's normal
    range; the 1/32 comes back out in the PSUM drain.  Q/Wo stay bf16
    (they feed residuals directly).
  * warm-up matmuls at t=0 so the PE HAM un-throttles while input DMAs
    stream, and DMA issue order puts Wq/X first (the baseline had the
    first matmul queued behind 8MB of X+Y).
  * the LN1 -> Wo -> LN2 tail is processed in two 512-token chunks so the
    DVE work of one chunk overlaps the PE work of the other (the baseline
    tail was DVE-bound with the PE idle and HAM oscillating).
"""

import math
import numpy as np

import concourse.bass as bass
import concourse.mybir as mybir
import concourse.tile as tile
from concourse import bacc
from concourse.bass_utils import run_bass_kernel_spmd

P = 128
NX = 1024
NY = 1024
DIM = 1024
H = 8
KO = DIM // P          # 8 partition sub-tiles of the model dim
QC = 512
NQC = NX // QC         # 2
F32 = mybir.dt.float32
BF16 = mybir.dt.bfloat16
FP8 = mybir.dt.float8e4
AF = mybir.ActivationFunctionType
ALU = mybir.AluOpType
DR = mybir.MatmulPerfMode.DoubleRow
SCALE = 1.0 / 32.0     # 1/sqrt(DIM)
EPS = 1e-5
USE_FP8 = True
W_PRESCALE = 32.0      # host multiplies Wk/Wv by this when USE_FP8
N_WARMUP = 24          # warm-up matmuls at t=0 (HAM un-throttle + DMA overlap)


def _build():
    nc = bacc.Bacc("TRN2", target_bir_lowering=False, debug=False,
                   enable_asserts=False)
    kvdt = FP8 if USE_FP8 else BF16
    kv_scale = (1.0 / W_PRESCALE) if USE_FP8 else 1.0

    # ---- DRAM I/O (per-core shapes) ----
    XT = nc.dram_tensor("XT", [DIM, NX], BF16, kind="ExternalInput").ap()
    YT = nc.dram_tensor("YT", [DIM, NY], kvdt, kind="ExternalInput").ap()
    MB = nc.dram_tensor("MB", [NY], F32, kind="ExternalInput").ap()
    WQ = nc.dram_tensor("Wq", [DIM, DIM], BF16, kind="ExternalInput").ap()
    WK = nc.dram_tensor("Wk", [DIM, DIM], kvdt, kind="ExternalInput").ap()
    WV = nc.dram_tensor("Wv", [DIM, DIM], kvdt, kind="ExternalInput").ap()
    WO = nc.dram_tensor("Wo", [DIM, DIM], BF16, kind="ExternalInput").ap()
    Vecs = {}
    for vname in ("bq", "bk", "bv", "bo", "g1", "b1", "g2", "b2"):
        Vecs[vname] = nc.dram_tensor(vname, [DIM], F32, kind="ExternalInput").ap()
    OT = nc.dram_tensor("OT", [DIM, NX], F32, kind="ExternalOutput").ap()

    xt3 = XT.rearrange("(ko p) q -> p ko q", p=P)
    yt3 = YT.rearrange("(ko p) q -> p ko q", p=P)
    wq3 = WQ.rearrange("(ko p) d -> p ko d", p=P)
    wk3 = WK.rearrange("(ko p) d -> p ko d", p=P)
    wv3 = WV.rearrange("(ko p) d -> p ko d", p=P)
    wo3 = WO.rearrange("(ko p) d -> p ko d", p=P)
    ot3 = OT.rearrange("(do p) q -> p do q", p=P)

    with tile.TileContext(nc) as tc:
        with tc.tile_pool(name="const", bufs=1) as const, \
             tc.tile_pool(name="act", bufs=1) as actp:

            # ---- constants ----
            ones_bf = const.tile([P, P], BF16, tag="onesbf", name="ones_bf")
            nc.vector.memset(ones_bf, 1.0)
            if USE_FP8:
                ones_f8 = const.tile([P, 2, P], FP8, tag="ones8", name="ones_f8")
                nc.vector.memset(ones_f8, 1.0)
            warm_rhs = const.tile([P, QC], BF16, tag="warm", name="warm_rhs")
            nc.vector.memset(warm_rhs, 0.0)
            eps_sb = const.tile([P, 1], F32, tag="eps", name="eps_sb")
            nc.vector.memset(eps_sb, EPS)

            # ---- PE warm-up: no input deps, keeps PE busy from t=0 ----
            with tc.tile_pool(name="warmp", bufs=2, space="PSUM") as wps:
                wp0 = wps.tile([P, QC], F32, tag="wps", name="warm_ps0")
                wp1 = wps.tile([P, QC], F32, tag="wps", name="warm_ps1")
                for i in range(N_WARMUP):
                    nc.tensor.matmul(wp0 if i % 2 == 0 else wp1,
                                     lhsT=ones_bf, rhs=warm_rhs,
                                     start=True, stop=True)

            def vec_pko(name):
                t = const.tile([P, KO], F32, tag=f"v_{name}", name=f"{name}_sb")
                nc.sync.dma_start(t, Vecs[name].rearrange("(ko p) -> p ko", p=P))
                return t

            mb_sb = const.tile([P, KO], F32, tag="v_mb", name="mb_sb")
            nc.sync.dma_start(mb_sb, MB.rearrange("(ko p) -> p ko", p=P))
            bq_sb = vec_pko("bq")
            bk_sb = vec_pko("bk")
            # bv is folded into the attention residual instead of into V:
            # softmax rows sum to 1, so A @ (V + 1 bv^T) = A @ V + bv.
            bv_sb = vec_pko("bv")
            bo_sb = vec_pko("bo")
            g1_sb = vec_pko("g1")
            b1_sb = vec_pko("b1")
            g2_sb = vec_pko("g2")
            b2_sb = vec_pko("b2")

            # ---- persistent feature-major activation tiles ----
            qt = actp.tile([P, KO, NX], BF16, tag="qt", name="qt")
            ktm = actp.tile([P, KO, NY], BF16, tag="ktm", name="ktm")
            vm = actp.tile([P, KO, DIM], kvdt, tag="vm", name="vm")
            zt = actp.tile([P, KO, NX], BF16, tag="zt", name="zt")
            o1t = actp.tile([P, KO, NX], BF16, tag="o1t", name="o1t")
            z2t = actp.tile([P, KO, NX], BF16, tag="z2t", name="z2t")
            wo_sb = actp.tile([P, KO, DIM], BF16, tag="wo", name="wo_sb")

            # ================= Phase 1: Q, K, V projections =================
            # PSUM budget: tag "ps" ([P,NX]=4KB) x2 bufs + tag "psv"
            # ([P,QC]=2KB) x4 bufs = 16KB exactly.
            with tc.tile_pool(name="io", bufs=1) as iop, \
                 tc.tile_pool(name="gp1", bufs=2, space="PSUM") as pp:
                xt = iop.tile([P, KO, NX], BF16, tag="xt", name="xt")
                yt = iop.tile([P, KO, NY], kvdt, tag="yt", name="yt")
                wq_sb = iop.tile([P, KO, DIM], BF16, tag="wq", name="wq_sb")
                wk_sb = iop.tile([P, KO, DIM], kvdt, tag="wk", name="wk_sb")
                wv_sb = iop.tile([P, KO, DIM], kvdt, tag="wv", name="wv_sb")

                # DMA issue order = need order: Wq(dg0)/X interleaved first,
                # then Wq(dg1), then Y/Wk/Wv, then Wo (tail).
                for k in range(KO):
                    nc.sync.dma_start(wq_sb[:, k, 0:QC], wq3[:, k, 0:QC])
                    nc.sync.dma_start(xt[:, k, :], xt3[:, k, :])
                for k in range(KO):
                    nc.sync.dma_start(wq_sb[:, k, QC:DIM], wq3[:, k, QC:DIM])
                for k in range(KO):
                    nc.sync.dma_start(yt[:, k, :], yt3[:, k, :])
                for k in range(KO):
                    nc.sync.dma_start(wk_sb[:, k, :], wk3[:, k, :])
                for k in range(KO):
                    nc.sync.dma_start(wv_sb[:, k, :], wv3[:, k, :])
                for k in range(KO):
                    nc.sync.dma_start(wo_sb[:, k, :], wo3[:, k, :])

                # --- Q projection (bf16); matmul out free dim <= 512 ---
                for dg in range(2):
                    for d4 in range(4):
                        do = dg * 4 + d4
                        ps = pp.tile([P, NX], F32, tag="ps", name=f"ps_q{do}")
                        for k in range(KO):
                            for qc in range(NQC):
                                qs = slice(qc * QC, (qc + 1) * QC)
                                nc.tensor.matmul(
                                    ps[:, qs],
                                    lhsT=wq_sb[:, k, do * P:(do + 1) * P],
                                    rhs=xt[:, k, qs],
                                    start=(k == 0), stop=(k == KO - 1))
                        nc.scalar.activation(
                            qt[:, do, :], ps, AF.Identity,
                            bias=bq_sb[:, do:do + 1], scale=1.0)

                # --- K projection ---
                for dg in range(2):
                    for d4 in range(4):
                        do = dg * 4 + d4
                        ps = pp.tile([P, NX], F32, tag="ps", name=f"ps_k{do}")
                        if USE_FP8:
                            for kp in range(KO // 2):
                                ks = slice(2 * kp, 2 * kp + 2)
                                for qc in range(NQC):
                                    qs = slice(qc * QC, (qc + 1) * QC)
                                    nc.tensor.matmul(
                                        ps[:, qs],
                                        lhsT=wk_sb[:, ks, do * P:(do + 1) * P],
                                        rhs=yt[:, ks, qs],
                                        start=(kp == 0), stop=(kp == KO // 2 - 1),
                                        perf_mode=DR)
                        else:
                            for k in range(KO):
                                for qc in range(NQC):
                                    qs = slice(qc * QC, (qc + 1) * QC)
                                    nc.tensor.matmul(
                                        ps[:, qs],
                                        lhsT=wk_sb[:, k, do * P:(do + 1) * P],
                                        rhs=yt[:, k, qs],
                                        start=(k == 0), stop=(k == KO - 1))
                        nc.scalar.activation(
                            ktm[:, do, :], ps, AF.Identity,
                            bias=bk_sb[:, do:do + 1], scale=kv_scale)

                # --- V projection, natural (token-major) layout ---
                # V[y, n] = sum_k Y[y, k] Wv[k, n]; bias added via a
                # partition-broadcast STT on the drain (DVE, idle in phase 1)
                for yo in range(KO):
                    pss = [pp.tile([P, QC], F32, tag="psv", name=f"ps_v{yo}{ng}",
                                   bufs=4)
                           for ng in range(2)]
                    if USE_FP8:
                        for kp in range(KO // 2):
                            ks = slice(2 * kp, 2 * kp + 2)
                            for ng in range(2):
                                nc.tensor.matmul(
                                    pss[ng],
                                    lhsT=yt[:, ks, yo * P:(yo + 1) * P],
                                    rhs=wv_sb[:, ks, ng * QC:(ng + 1) * QC],
                                    start=(kp == 0), stop=(kp == KO // 2 - 1),
                                    perf_mode=DR)
                    else:
                        for k in range(KO):
                            for ng in range(2):
                                nc.tensor.matmul(
                                    pss[ng],
                                    lhsT=yt[:, k, yo * P:(yo + 1) * P],
                                    rhs=wv_sb[:, k, ng * QC:(ng + 1) * QC],
                                    start=(k == 0), stop=(k == KO - 1))
                    for ng in range(2):
                        ns = slice(ng * QC, (ng + 1) * QC)
                        nc.vector.tensor_scalar_mul(vm[:, yo, ns], pss[ng],
                                                    kv_scale)

            # ================= Phase 2: attention =================
            with tc.tile_pool(name="exp", bufs=2) as ep, \
                 tc.tile_pool(name="rcp", bufs=2) as rp, \
                 tc.tile_pool(name="lgp", bufs=2, space="PSUM") as lgp, \
                 tc.tile_pool(name="avp", bufs=1, space="PSUM") as avp, \
                 tc.tile_pool(name="rlp", bufs=1, space="PSUM") as rlp:

                def logits_exp(h):
                    # logitsT[k, q] = sum_d KT_h[d, k] QT_h[d, q]; exp with
                    # per-key mask bias and the 1/32 scale; output goes
                    # straight to fp8/bf16 for the AV + denominator matmuls.
                    et = ep.tile([P, KO, NX], kvdt, tag="exp", name=f"et{h}")
                    for kt in range(KO):
                        pl = lgp.tile([P, NX], F32, tag="lg", name=f"pl{h}{kt}")
                        for qc in range(NQC):
                            qs = slice(qc * QC, (qc + 1) * QC)
                            nc.tensor.matmul(
                                pl[:, qs],
                                lhsT=ktm[:, h, kt * P:(kt + 1) * P],
                                rhs=qt[:, h, qs],
                                start=True, stop=True)
                        nc.scalar.activation(
                            et[:, kt, :], pl, AF.Exp,
                            bias=mb_sb[:, kt:kt + 1], scale=SCALE)
                    return et

                def denom_av(h, et):
                    # softmax denominator via all-ones matmul (partition
                    # reduction + broadcast in one shot), then AV, then
                    # normalize + Q residual.
                    pr = rlp.tile([P, NX], F32, tag="rl", name=f"pr{h}")
                    pa = avp.tile([P, NX], F32, tag="av", name=f"pa{h}")
                    if USE_FP8:
                        for kp in range(KO // 2):
                            ks = slice(2 * kp, 2 * kp + 2)
                            for qc in range(NQC):
                                qs = slice(qc * QC, (qc + 1) * QC)
                                nc.tensor.matmul(
                                    pr[:, qs], lhsT=ones_f8,
                                    rhs=et[:, ks, qs],
                                    start=(kp == 0), stop=(kp == KO // 2 - 1),
                                    perf_mode=DR)
                        for kp in range(KO // 2):
                            ks = slice(2 * kp, 2 * kp + 2)
                            for qc in range(NQC):
                                qs = slice(qc * QC, (qc + 1) * QC)
                                nc.tensor.matmul(
                                    pa[:, qs],
                                    lhsT=vm[:, ks, h * P:(h + 1) * P],
                                    rhs=et[:, ks, qs],
                                    start=(kp == 0), stop=(kp == KO // 2 - 1),
                                    perf_mode=DR)
                    else:
                        for kt in range(KO):
                            for qc in range(NQC):
                                qs = slice(qc * QC, (qc + 1) * QC)
                                nc.tensor.matmul(
                                    pr[:, qs], lhsT=ones_bf, rhs=et[:, kt, qs],
                                    start=(kt == 0), stop=(kt == KO - 1))
                        for kt in range(KO):
                            for qc in range(NQC):
                                qs = slice(qc * QC, (qc + 1) * QC)
                                nc.tensor.matmul(
                                    pa[:, qs],
                                    lhsT=vm[:, kt, h * P:(h + 1) * P],
                                    rhs=et[:, kt, qs],
                                    start=(kt == 0), stop=(kt == KO - 1))
                    rc = rp.tile([P, NX], F32, tag="rc", name=f"rc{h}")
                    nc.vector.reciprocal_approx_fast(rc, pr)
                    nc.vector.tensor_mul(zt[:, h, :], pa, rc)
                    # zt = (attn + bv) + qt  -- the bv fold (see consts)
                    nc.vector.scalar_tensor_tensor(
                        zt[:, h, :], zt[:, h, :], bv_sb[:, h:h + 1],
                        qt[:, h, :], op0=ALU.add, op1=ALU.add)

                # software pipeline: head h's logits/exp (PE+ACT) run while
                # head h-1's denominator+AV (PE) wait on h-1's exp
                prev = None
                for h in range(H):
                    et = logits_exp(h)
                    if prev is not None:
                        denom_av(h - 1, prev)
                    prev = et
                denom_av(H - 1, prev)

            # ========== Phase 3: LN1 -> Wo(+relu, residual) -> LN2 ==========
            # Processed in two 512-token chunks, program-ordered so one
            # chunk's DVE work overlaps the other chunk's PE work.
            # PSUM: tags "pmu"/"ps2" x2 bufs (8KB) + tag "ps" x4 (8KB) = 16KB
            with tc.tile_pool(name="sqp", bufs=6) as sqp, \
                 tc.tile_pool(name="stp", bufs=2) as stp, \
                 tc.tile_pool(name="outp", bufs=4) as outp, \
                 tc.tile_pool(name="spp", bufs=2, space="PSUM") as spp, \
                 tc.tile_pool(name="gp3", bufs=4, space="PSUM") as pp3:

                def ln_stats(in_sb, j, lbl):
                    # returns (rsig, mrs) for token chunk j
                    qs = slice(j * QC, (j + 1) * QC)
                    pmu = spp.tile([P, QC], F32, tag="pmu", name=f"pmu{lbl}{j}")
                    ps2 = spp.tile([P, QC], F32, tag="ps2", name=f"ps2{lbl}{j}")
                    for do in range(KO):
                        nc.tensor.matmul(pmu, lhsT=ones_bf,
                                         rhs=in_sb[:, do, qs],
                                         start=(do == 0), stop=(do == KO - 1))
                    for do in range(KO):
                        sq = sqp.tile([P, QC], BF16, tag="sq",
                                      name=f"sq{lbl}{j}{do}")
                        nc.vector.tensor_mul(sq, in_sb[:, do, qs],
                                             in_sb[:, do, qs])
                        nc.tensor.matmul(ps2, lhsT=ones_bf, rhs=sq,
                                         start=(do == 0), stop=(do == KO - 1))
                    mu = stp.tile([P, QC], F32, tag="mu", name=f"mu{lbl}{j}")
                    nc.vector.tensor_scalar_mul(mu, pmu, 1.0 / DIM)
                    msq = stp.tile([P, QC], F32, tag="msq", name=f"msq{lbl}{j}")
                    nc.vector.tensor_mul(msq, mu, mu)
                    sd = stp.tile([P, QC], F32, tag="sd", name=f"sd{lbl}{j}")
                    nc.vector.scalar_tensor_tensor(
                        sd, ps2, 1.0 / DIM, msq,
                        op0=ALU.mult, op1=ALU.subtract)
                    nc.scalar.activation(sd, sd, AF.Sqrt, bias=eps_sb, scale=1.0)
                    rsig = stp.tile([P, QC], F32, tag="rsig",
                                    name=f"rsig{lbl}{j}")
                    nc.vector.reciprocal_approx_fast(rsig, sd)
                    mrs = stp.tile([P, QC], F32, tag="mrs", name=f"mrs{lbl}{j}")
                    nc.vector.tensor_mul(mrs, mu, rsig)
                    return rsig, mrs

                def ln_emit(in_sb, j, rsig, mrs, g_sb, b_sb, emit):
                    qs = slice(j * QC, (j + 1) * QC)
                    for do in range(KO):
                        t = sqp.tile([P, QC], F32, tag="t", name=f"t{j}{do}")
                        nc.vector.tensor_mul(t, in_sb[:, do, qs], rsig)
                        nc.vector.tensor_sub(t, t, mrs)
                        emit(do, qs, t)

                def emit_o1(do, qs, t):
                    nc.vector.tensor_scalar(
                        o1t[:, do, qs], t,
                        scalar1=g1_sb[:, do:do + 1],
                        scalar2=b1_sb[:, do:do + 1],
                        op0=ALU.mult, op1=ALU.add)

                def wo_proj(j):
                    qs = slice(j * QC, (j + 1) * QC)
                    for ng in range(2):
                        for n4 in range(4):
                            no = ng * 4 + n4
                            ps = pp3.tile([P, QC], F32, tag="ps",
                                          name=f"ps_o{j}{no}")
                            for k in range(KO):
                                nc.tensor.matmul(
                                    ps,
                                    lhsT=wo_sb[:, k, no * P:(no + 1) * P],
                                    rhs=o1t[:, k, qs],
                                    start=(k == 0), stop=(k == KO - 1))
                            ht = sqp.tile([P, QC], BF16, tag="ht",
                                          name=f"ht{j}{no}")
                            nc.scalar.activation(ht, ps, AF.Relu,
                                                 bias=bo_sb[:, no:no + 1],
                                                 scale=1.0)
                            nc.vector.tensor_add(z2t[:, no, qs], ht,
                                                 o1t[:, no, qs])

                def emit_o2(do, qs, t):
                    o = outp.tile([P, QC], F32, tag="o", name=f"o{do}")
                    nc.vector.tensor_scalar(
                        o, t,
                        scalar1=g2_sb[:, do:do + 1],
                        scalar2=b2_sb[:, do:do + 1],
                        op0=ALU.mult, op1=ALU.add)
                    nc.sync.dma_start(ot3[:, do, qs], o)

                # chunk-interleaved program order for PE/DVE overlap
                s10 = ln_stats(zt, 0, "a")
                s11 = ln_stats(zt, 1, "a")
                ln_emit(zt, 0, *s10, g1_sb, b1_sb, emit_o1)
                wo_proj(0)
                ln_emit(zt, 1, *s11, g1_sb, b1_sb, emit_o1)
                wo_proj(1)
                s20 = ln_stats(z2t, 0, "b")
                ln_emit(z2t, 0, *s20, g2_sb, b2_sb, emit_o2)
                s21 = ln_stats(z2t, 1, "b")
                ln_emit(z2t, 1, *s21, g2_sb, b2_sb, emit_o2)

    nc.compile()
    return nc


_CACHE = {}


def _get_nc():
    if "nc" not in _CACHE:
        _CACHE["nc"] = _build()
    return _CACHE["nc"]


def make_in_maps(X, Y, mask, Wq, bq, Wk, bk, Wv, bv, Wo, bo, g1, b1, g2, b2):
    import ml_dtypes
    bf16 = ml_dtypes.bfloat16
    f8 = ml_dtypes.float8_e4m3
    kvdt = f8 if USE_FP8 else bf16
    wsc = np.float32(W_PRESCALE) if USE_FP8 else np.float32(1.0)

    f32 = lambda a: np.ascontiguousarray(np.asarray(a, dtype=np.float32))
    shared = {
        "Wq": np.ascontiguousarray(np.asarray(Wq, np.float32).astype(bf16)),
        "Wk": np.ascontiguousarray(
            (np.asarray(Wk, np.float32) * wsc).astype(kvdt)),
        "Wv": np.ascontiguousarray(
            (np.asarray(Wv, np.float32) * wsc).astype(kvdt)),
        "Wo": np.ascontiguousarray(np.asarray(Wo, np.float32).astype(bf16)),
        "bq": f32(bq), "bk": f32(bk), "bv": f32(bv), "bo": f32(bo),
        "g1": f32(g1), "b1": f32(b1), "g2": f32(g2), "b2": f32(b2),
    }
    X = np.asarray(X, dtype=np.float32)
    Y = np.asarray(Y, dtype=np.float32)
    mask = np.asarray(mask)
    in_maps = []
    for b in range(8):
        mb = np.where(mask[b], np.float32(-1e4), np.float32(0.0)).astype(np.float32)
        in_maps.append({
            "XT": np.ascontiguousarray(X[b].T.astype(bf16)),
            "YT": np.ascontiguousarray(Y[b].T.astype(kvdt)),
            "MB": mb,
            **shared,
        })
    return in_maps


def kernel(X, Y, mask, Wq, bq, Wk, bk, Wv, bv, Wo, bo, g1, b1, g2, b2,
           _trace=False):
    nc = _get_nc()
    in_maps = make_in_maps(X, Y, mask, Wq, bq, Wk, bk, Wv, bv, Wo, bo,
                           g1, b1, g2, b2)
    res = run_bass_kernel_spmd(nc, in_maps, core_ids=list(range(8)),
                               trace=_trace)
    out = np.stack([np.ascontiguousarray(res.results[b]["OT"].T)
                    for b in range(8)]).astype(np.float32)
    if _trace:
        return out, res
    return out


# revision 21
# speedup vs baseline: 1.5219x; 1.0417x over previous
"""Trainium2 Bass kernel for a masked-attention block (MAB).

Computation (per batch element):
    Q = X@Wq + bq ; K = Y@Wk + bk ; V = Y@Wv + bv
    logits = per-head Qh@Kh^T / 32, masked keys -> -inf, softmax over keys
    attn   = A @ Vh (concat heads)
    O1 = LN(Q + attn; g1,b1)
    O  = LN(O1 + relu(O1@Wo + bo); g2,b2)

Sharding: pure data-parallel, one batch element per NeuronCore (B=8 = 8 cores).

On-device dataflow is "feature-major": activations live in SBUF transposed
([model_dim -> 8x128 partitions, token -> free]) so every matmul chains
without transposes.  Softmax denominators and LayerNorm stats are
partition-dim reductions done with all-ones stationary matmuls (which also
broadcast the result across partitions for free).

v2 changes vs the f32 baseline:
  * bf16 activations/weights end to end (same PE rate, 2x DVE rate, half
    the DMA bytes).  Host converts on the way in; output stays f32.
  * fp8(e4m3) + DoubleRow for the K/V projections, the AV matmul and the
    softmax denominator (2 contraction-tiles per matmul -> ~2x PE).  Wk/Wv
    are pre-scaled by 32 on the host so their values sit in e4m3's normal
    range; the 1/32 comes back out in the PSUM drain.  Q/Wo stay bf16
    (they feed residuals directly).
  * warm-up matmuls at t=0 so the PE HAM un-throttles while input DMAs
    stream, and DMA issue order puts Wq/X first (the baseline had the
    first matmul queued behind 8MB of X+Y).
  * the LN1 -> Wo -> LN2 tail is processed in two 512-token chunks so the
    DVE work of one chunk overlaps the PE work of the other (the baseline
    tail was DVE-bound with the PE idle and HAM oscillating).
"""

import math
import numpy as np

import concourse.bass as bass
import concourse.mybir as mybir
import concourse.tile as tile
from concourse import bacc
from concourse.bass_utils import run_bass_kernel_spmd

P = 128
NX = 1024
NY = 1024
DIM = 1024
H = 8
KO = DIM // P          # 8 partition sub-tiles of the model dim
QC = 512
NQC = NX // QC         # 2
F32 = mybir.dt.float32
BF16 = mybir.dt.bfloat16
FP8 = mybir.dt.float8e4
AF = mybir.ActivationFunctionType
ALU = mybir.AluOpType
DR = mybir.MatmulPerfMode.DoubleRow
SCALE = 1.0 / 32.0     # 1/sqrt(DIM)
EPS = 1e-5
USE_FP8 = True
W_PRESCALE = 32.0      # host multiplies Wk/Wv by this when USE_FP8
N_WARMUP = 24          # warm-up matmuls at t=0 (HAM un-throttle + DMA overlap)


def _build():
    nc = bacc.Bacc("TRN2", target_bir_lowering=False, debug=False,
                   enable_asserts=False)
    kvdt = FP8 if USE_FP8 else BF16
    kv_scale = (1.0 / W_PRESCALE) if USE_FP8 else 1.0

    # ---- DRAM I/O (per-core shapes) ----
    XT = nc.dram_tensor("XT", [DIM, NX], BF16, kind="ExternalInput").ap()
    YT = nc.dram_tensor("YT", [DIM, NY], kvdt, kind="ExternalInput").ap()
    MB = nc.dram_tensor("MB", [NY], F32, kind="ExternalInput").ap()
    WQ = nc.dram_tensor("Wq", [DIM, DIM], BF16, kind="ExternalInput").ap()
    WK = nc.dram_tensor("Wk", [DIM, DIM], kvdt, kind="ExternalInput").ap()
    WV = nc.dram_tensor("Wv", [DIM, DIM], kvdt, kind="ExternalInput").ap()
    WO = nc.dram_tensor("Wo", [DIM, DIM], BF16, kind="ExternalInput").ap()
    Vecs = {}
    for vname in ("bq", "bk", "bv", "bo", "g1", "b1", "g2", "b2"):
        Vecs[vname] = nc.dram_tensor(vname, [DIM], F32, kind="ExternalInput").ap()
    OT = nc.dram_tensor("OT", [DIM, NX], BF16, kind="ExternalOutput").ap()

    xt3 = XT.rearrange("(ko p) q -> p ko q", p=P)
    yt3 = YT.rearrange("(ko p) q -> p ko q", p=P)
    wq3 = WQ.rearrange("(ko p) d -> p ko d", p=P)
    wk3 = WK.rearrange("(ko p) d -> p ko d", p=P)
    wv3 = WV.rearrange("(ko p) d -> p ko d", p=P)
    wo3 = WO.rearrange("(ko p) d -> p ko d", p=P)
    ot3 = OT.rearrange("(do p) q -> p do q", p=P)

    with tile.TileContext(nc) as tc:
        with tc.tile_pool(name="const", bufs=1) as const, \
             tc.tile_pool(name="act", bufs=1) as actp:

            # ---- constants ----
            ones_bf = const.tile([P, P], BF16, tag="onesbf", name="ones_bf")
            nc.vector.memset(ones_bf, 1.0)
            if USE_FP8:
                ones_f8 = const.tile([P, 2, P], FP8, tag="ones8", name="ones_f8")
                nc.vector.memset(ones_f8, 1.0)
            warm_rhs = const.tile([P, QC], BF16, tag="warm", name="warm_rhs")
            nc.vector.memset(warm_rhs, 0.0)
            eps_sb = const.tile([P, 1], F32, tag="eps", name="eps_sb")
            nc.vector.memset(eps_sb, EPS)

            # ---- PE warm-up: no input deps, keeps PE busy from t=0 ----
            with tc.tile_pool(name="warmp", bufs=2, space="PSUM") as wps:
                wp0 = wps.tile([P, QC], F32, tag="wps", name="warm_ps0")
                wp1 = wps.tile([P, QC], F32, tag="wps", name="warm_ps1")
                for i in range(N_WARMUP):
                    nc.tensor.matmul(wp0 if i % 2 == 0 else wp1,
                                     lhsT=ones_bf, rhs=warm_rhs,
                                     start=True, stop=True)

            def vec_pko(name):
                t = const.tile([P, KO], F32, tag=f"v_{name}", name=f"{name}_sb")
                nc.sync.dma_start(t, Vecs[name].rearrange("(ko p) -> p ko", p=P))
                return t

            mb_sb = const.tile([P, KO], F32, tag="v_mb", name="mb_sb")
            nc.sync.dma_start(mb_sb, MB.rearrange("(ko p) -> p ko", p=P))
            bq_sb = vec_pko("bq")
            bk_sb = vec_pko("bk")
            # bv is folded into the attention residual instead of into V:
            # softmax rows sum to 1, so A @ (V + 1 bv^T) = A @ V + bv.
            bv_sb = vec_pko("bv")
            bo_sb = vec_pko("bo")
            g1_sb = vec_pko("g1")
            b1_sb = vec_pko("b1")
            g2_sb = vec_pko("g2")
            b2_sb = vec_pko("b2")

            # ---- persistent feature-major activation tiles ----
            qt = actp.tile([P, KO, NX], BF16, tag="qt", name="qt")
            ktm = actp.tile([P, KO, NY], BF16, tag="ktm", name="ktm")
            vm = actp.tile([P, KO, DIM], kvdt, tag="vm", name="vm")
            zt = actp.tile([P, KO, NX], BF16, tag="zt", name="zt")
            o1t = actp.tile([P, KO, NX], BF16, tag="o1t", name="o1t")
            z2t = actp.tile([P, KO, NX], BF16, tag="z2t", name="z2t")
            wo_sb = actp.tile([P, KO, DIM], BF16, tag="wo", name="wo_sb")

            # ================= Phase 1: Q, K, V projections =================
            # PSUM budget: tag "ps" ([P,NX]=4KB) x2 bufs + tag "psv"
            # ([P,QC]=2KB) x4 bufs = 16KB exactly.
            with tc.tile_pool(name="io", bufs=1) as iop, \
                 tc.tile_pool(name="gp1", bufs=2, space="PSUM") as pp:
                xt = iop.tile([P, KO, NX], BF16, tag="xt", name="xt")
                yt = iop.tile([P, KO, NY], kvdt, tag="yt", name="yt")
                wq_sb = iop.tile([P, KO, DIM], BF16, tag="wq", name="wq_sb")
                wk_sb = iop.tile([P, KO, DIM], kvdt, tag="wk", name="wk_sb")
                wv_sb = iop.tile([P, KO, DIM], kvdt, tag="wv", name="wv_sb")

                # DMA issue order = need order: Wq(dg0)/X interleaved first,
                # then Wq(dg1), then Y/Wk/Wv, then Wo (tail).
                for k in range(KO):
                    nc.sync.dma_start(wq_sb[:, k, 0:QC], wq3[:, k, 0:QC])
                    nc.sync.dma_start(xt[:, k, :], xt3[:, k, :])
                for k in range(KO):
                    nc.sync.dma_start(wq_sb[:, k, QC:DIM], wq3[:, k, QC:DIM])
                for k in range(KO):
                    nc.sync.dma_start(yt[:, k, :], yt3[:, k, :])
                for k in range(KO):
                    nc.sync.dma_start(wk_sb[:, k, :], wk3[:, k, :])
                for k in range(KO):
                    nc.sync.dma_start(wv_sb[:, k, :], wv3[:, k, :])
                for k in range(KO):
                    nc.sync.dma_start(wo_sb[:, k, :], wo3[:, k, :])

                # --- Q projection (bf16); matmul out free dim <= 512 ---
                for dg in range(2):
                    for d4 in range(4):
                        do = dg * 4 + d4
                        ps = pp.tile([P, NX], F32, tag="ps", name=f"ps_q{do}")
                        for k in range(KO):
                            for qc in range(NQC):
                                qs = slice(qc * QC, (qc + 1) * QC)
                                nc.tensor.matmul(
                                    ps[:, qs],
                                    lhsT=wq_sb[:, k, do * P:(do + 1) * P],
                                    rhs=xt[:, k, qs],
                                    start=(k == 0), stop=(k == KO - 1))
                        nc.scalar.activation(
                            qt[:, do, :], ps, AF.Identity,
                            bias=bq_sb[:, do:do + 1], scale=1.0)

                # --- K projection ---
                for dg in range(2):
                    for d4 in range(4):
                        do = dg * 4 + d4
                        ps = pp.tile([P, NX], F32, tag="ps", name=f"ps_k{do}")
                        if USE_FP8:
                            for kp in range(KO // 2):
                                ks = slice(2 * kp, 2 * kp + 2)
                                for qc in range(NQC):
                                    qs = slice(qc * QC, (qc + 1) * QC)
                                    nc.tensor.matmul(
                                        ps[:, qs],
                                        lhsT=wk_sb[:, ks, do * P:(do + 1) * P],
                                        rhs=yt[:, ks, qs],
                                        start=(kp == 0), stop=(kp == KO // 2 - 1),
                                        perf_mode=DR)
                        else:
                            for k in range(KO):
                                for qc in range(NQC):
                                    qs = slice(qc * QC, (qc + 1) * QC)
                                    nc.tensor.matmul(
                                        ps[:, qs],
                                        lhsT=wk_sb[:, k, do * P:(do + 1) * P],
                                        rhs=yt[:, k, qs],
                                        start=(k == 0), stop=(k == KO - 1))
                        # drain on DVE (keeps ACT free for the exp stream)
                        nc.vector.tensor_scalar(
                            ktm[:, do, :], ps,
                            scalar1=kv_scale,
                            scalar2=bk_sb[:, do:do + 1],
                            op0=ALU.mult, op1=ALU.add)

                # --- V projection, natural (token-major) layout ---
                # V[y, n] = sum_k Y[y, k] Wv[k, n]; bias added via a
                # partition-broadcast STT on the drain (DVE, idle in phase 1)
                for yo in range(KO):
                    pss = [pp.tile([P, QC], F32, tag="psv", name=f"ps_v{yo}{ng}",
                                   bufs=4)
                           for ng in range(2)]
                    if USE_FP8:
                        for kp in range(KO // 2):
                            ks = slice(2 * kp, 2 * kp + 2)
                            for ng in range(2):
                                nc.tensor.matmul(
                                    pss[ng],
                                    lhsT=yt[:, ks, yo * P:(yo + 1) * P],
                                    rhs=wv_sb[:, ks, ng * QC:(ng + 1) * QC],
                                    start=(kp == 0), stop=(kp == KO // 2 - 1),
                                    perf_mode=DR)
                    else:
                        for k in range(KO):
                            for ng in range(2):
                                nc.tensor.matmul(
                                    pss[ng],
                                    lhsT=yt[:, k, yo * P:(yo + 1) * P],
                                    rhs=wv_sb[:, k, ng * QC:(ng + 1) * QC],
                                    start=(k == 0), stop=(k == KO - 1))
                    for ng in range(2):
                        ns = slice(ng * QC, (ng + 1) * QC)
                        nc.vector.tensor_scalar_mul(vm[:, yo, ns], pss[ng],
                                                    kv_scale)

            # ================= Phase 2: attention =================
            with tc.tile_pool(name="exp", bufs=2) as ep, \
                 tc.tile_pool(name="rcp", bufs=2) as rp, \
                 tc.tile_pool(name="lgp", bufs=2, space="PSUM") as lgp, \
                 tc.tile_pool(name="avp", bufs=1, space="PSUM") as avp, \
                 tc.tile_pool(name="rlp", bufs=1, space="PSUM") as rlp:

                def logits_exp(h):
                    # logitsT[k, q] = sum_d KT_h[d, k] QT_h[d, q]; exp with
                    # per-key mask bias and the 1/32 scale; output goes
                    # straight to fp8/bf16 for the AV + denominator matmuls.
                    et = ep.tile([P, KO, NX], kvdt, tag="exp", name=f"et{h}")
                    for kt in range(KO):
                        pl = lgp.tile([P, NX], F32, tag="lg", name=f"pl{h}{kt}")
                        for qc in range(NQC):
                            qs = slice(qc * QC, (qc + 1) * QC)
                            nc.tensor.matmul(
                                pl[:, qs],
                                lhsT=ktm[:, h, kt * P:(kt + 1) * P],
                                rhs=qt[:, h, qs],
                                start=True, stop=True)
                        nc.scalar.activation(
                            et[:, kt, :], pl, AF.Exp,
                            bias=mb_sb[:, kt:kt + 1], scale=SCALE)
                    return et

                def denom_av(h, et):
                    # softmax denominator via all-ones matmul (partition
                    # reduction + broadcast in one shot), then AV, then
                    # normalize + Q residual.
                    pr = rlp.tile([P, NX], F32, tag="rl", name=f"pr{h}")
                    pa = avp.tile([P, NX], F32, tag="av", name=f"pa{h}")
                    if USE_FP8:
                        for kp in range(KO // 2):
                            ks = slice(2 * kp, 2 * kp + 2)
                            for qc in range(NQC):
                                qs = slice(qc * QC, (qc + 1) * QC)
                                nc.tensor.matmul(
                                    pr[:, qs], lhsT=ones_f8,
                                    rhs=et[:, ks, qs],
                                    start=(kp == 0), stop=(kp == KO // 2 - 1),
                                    perf_mode=DR)
                        for kp in range(KO // 2):
                            ks = slice(2 * kp, 2 * kp + 2)
                            for qc in range(NQC):
                                qs = slice(qc * QC, (qc + 1) * QC)
                                nc.tensor.matmul(
                                    pa[:, qs],
                                    lhsT=vm[:, ks, h * P:(h + 1) * P],
                                    rhs=et[:, ks, qs],
                                    start=(kp == 0), stop=(kp == KO // 2 - 1),
                                    perf_mode=DR)
                    else:
                        for kt in range(KO):
                            for qc in range(NQC):
                                qs = slice(qc * QC, (qc + 1) * QC)
                                nc.tensor.matmul(
                                    pr[:, qs], lhsT=ones_bf, rhs=et[:, kt, qs],
                                    start=(kt == 0), stop=(kt == KO - 1))
                        for kt in range(KO):
                            for qc in range(NQC):
                                qs = slice(qc * QC, (qc + 1) * QC)
                                nc.tensor.matmul(
                                    pa[:, qs],
                                    lhsT=vm[:, kt, h * P:(h + 1) * P],
                                    rhs=et[:, kt, qs],
                                    start=(kt == 0), stop=(kt == KO - 1))
                    rc = rp.tile([P, NX], F32, tag="rc", name=f"rc{h}")
                    nc.vector.reciprocal_approx_fast(rc, pr)
                    nc.vector.tensor_mul(zt[:, h, :], pa, rc)
                    # zt = (attn + bv) + qt  -- the bv fold (see consts)
                    nc.vector.scalar_tensor_tensor(
                        zt[:, h, :], zt[:, h, :], bv_sb[:, h:h + 1],
                        qt[:, h, :], op0=ALU.add, op1=ALU.add)

                # software pipeline: head h's logits/exp (PE+ACT) run while
                # head h-1's denominator+AV (PE) wait on h-1's exp
                prev = None
                for h in range(H):
                    et = logits_exp(h)
                    if prev is not None:
                        denom_av(h - 1, prev)
                    prev = et
                denom_av(H - 1, prev)

            # ========== Phase 3: LN1 -> Wo(+relu, residual) -> LN2 ==========
            # Processed in two 512-token chunks, program-ordered so one
            # chunk's DVE work overlaps the other chunk's PE work.
            # PSUM: tags "pmu"/"ps2" x2 bufs (8KB) + tag "ps" x4 (8KB) = 16KB
            with tc.tile_pool(name="sqp", bufs=6) as sqp, \
                 tc.tile_pool(name="stp", bufs=2) as stp, \
                 tc.tile_pool(name="outp", bufs=4) as outp, \
                 tc.tile_pool(name="spp", bufs=2, space="PSUM") as spp, \
                 tc.tile_pool(name="gp3", bufs=4, space="PSUM") as pp3:

                def ln_stats(in_sb, j, lbl):
                    # returns (rsig, mrs) as bf16 tiles (so the emit TTs run
                    # in the DVE's packed 2x mode) for token chunk j
                    qs = slice(j * QC, (j + 1) * QC)
                    pmu = spp.tile([P, QC], F32, tag="pmu", name=f"pmu{lbl}{j}")
                    ps2 = spp.tile([P, QC], F32, tag="ps2", name=f"ps2{lbl}{j}")
                    for do in range(KO):
                        nc.tensor.matmul(pmu, lhsT=ones_bf,
                                         rhs=in_sb[:, do, qs],
                                         start=(do == 0), stop=(do == KO - 1))
                    for do in range(KO):
                        sq = sqp.tile([P, QC], BF16, tag="sq",
                                      name=f"sq{lbl}{j}{do}")
                        nc.scalar.square(sq, in_sb[:, do, qs])
                        nc.tensor.matmul(ps2, lhsT=ones_bf, rhs=sq,
                                         start=(do == 0), stop=(do == KO - 1))
                    mu = stp.tile([P, QC], F32, tag="mu", name=f"mu{lbl}{j}")
                    nc.vector.tensor_scalar_mul(mu, pmu, 1.0 / DIM)
                    msq = stp.tile([P, QC], F32, tag="msq", name=f"msq{lbl}{j}")
                    nc.vector.tensor_mul(msq, mu, mu)
                    sd = stp.tile([P, QC], F32, tag="sd", name=f"sd{lbl}{j}")
                    nc.vector.scalar_tensor_tensor(
                        sd, ps2, 1.0 / DIM, msq,
                        op0=ALU.mult, op1=ALU.subtract)
                    nc.scalar.activation(sd, sd, AF.Sqrt, bias=eps_sb, scale=1.0)
                    rsf = stp.tile([P, QC], F32, tag="rsf", name=f"rsf{lbl}{j}")
                    nc.vector.reciprocal_approx_fast(rsf, sd)
                    rsig = stp.tile([P, QC], BF16, tag="rsig",
                                    name=f"rsig{lbl}{j}")
                    nc.vector.tensor_copy(rsig, rsf)
                    mrs = stp.tile([P, QC], BF16, tag="mrs", name=f"mrs{lbl}{j}")
                    nc.vector.tensor_mul(mrs, mu, rsf)
                    return rsig, mrs

                def ln_emit(in_sb, j, rsig, mrs, g_sb, b_sb, emit):
                    qs = slice(j * QC, (j + 1) * QC)
                    for do in range(KO):
                        t = sqp.tile([P, QC], BF16, tag="t", name=f"t{j}{do}")
                        nc.vector.tensor_mul(t, in_sb[:, do, qs], rsig)
                        nc.vector.tensor_sub(t, t, mrs)
                        emit(do, qs, t)

                def emit_o1(do, qs, t):
                    nc.vector.tensor_scalar(
                        o1t[:, do, qs], t,
                        scalar1=g1_sb[:, do:do + 1],
                        scalar2=b1_sb[:, do:do + 1],
                        op0=ALU.mult, op1=ALU.add)

                def wo_proj(j):
                    qs = slice(j * QC, (j + 1) * QC)
                    for ng in range(2):
                        for n4 in range(4):
                            no = ng * 4 + n4
                            ps = pp3.tile([P, QC], F32, tag="ps",
                                          name=f"ps_o{j}{no}")
                            for k in range(KO):
                                nc.tensor.matmul(
                                    ps,
                                    lhsT=wo_sb[:, k, no * P:(no + 1) * P],
                                    rhs=o1t[:, k, qs],
                                    start=(k == 0), stop=(k == KO - 1))
                            ht = sqp.tile([P, QC], BF16, tag="ht",
                                          name=f"ht{j}{no}")
                            nc.scalar.activation(ht, ps, AF.Relu,
                                                 bias=bo_sb[:, no:no + 1],
                                                 scale=1.0)
                            # residual add on GpSimd (DVE is the tail's
                            # critical engine)
                            nc.gpsimd.tensor_add(z2t[:, no, qs], ht,
                                                 o1t[:, no, qs])

                def emit_o2(do, qs, t):
                    o = outp.tile([P, QC], BF16, tag="o", name=f"o{do}")
                    nc.vector.tensor_scalar(
                        o, t,
                        scalar1=g2_sb[:, do:do + 1],
                        scalar2=b2_sb[:, do:do + 1],
                        op0=ALU.mult, op1=ALU.add)
                    nc.sync.dma_start(ot3[:, do, qs], o)

                # chunk-interleaved program order for PE/DVE overlap
                s10 = ln_stats(zt, 0, "a")
                s11 = ln_stats(zt, 1, "a")
                ln_emit(zt, 0, *s10, g1_sb, b1_sb, emit_o1)
                wo_proj(0)
                ln_emit(zt, 1, *s11, g1_sb, b1_sb, emit_o1)
                wo_proj(1)
                s20 = ln_stats(z2t, 0, "b")
                ln_emit(z2t, 0, *s20, g2_sb, b2_sb, emit_o2)
                s21 = ln_stats(z2t, 1, "b")
                ln_emit(z2t, 1, *s21, g2_sb, b2_sb, emit_o2)

    nc.compile()
    return nc


_CACHE = {}


def _get_nc():
    if "nc" not in _CACHE:
        _CACHE["nc"] = _build()
    return _CACHE["nc"]


def make_in_maps(X, Y, mask, Wq, bq, Wk, bk, Wv, bv, Wo, bo, g1, b1, g2, b2):
    import ml_dtypes
    bf16 = ml_dtypes.bfloat16
    f8 = ml_dtypes.float8_e4m3
    kvdt = f8 if USE_FP8 else bf16
    wsc = np.float32(W_PRESCALE) if USE_FP8 else np.float32(1.0)

    f32 = lambda a: np.ascontiguousarray(np.asarray(a, dtype=np.float32))
    shared = {
        "Wq": np.ascontiguousarray(np.asarray(Wq, np.float32).astype(bf16)),
        "Wk": np.ascontiguousarray(
            (np.asarray(Wk, np.float32) * wsc).astype(kvdt)),
        "Wv": np.ascontiguousarray(
            (np.asarray(Wv, np.float32) * wsc).astype(kvdt)),
        "Wo": np.ascontiguousarray(np.asarray(Wo, np.float32).astype(bf16)),
        "bq": f32(bq), "bk": f32(bk), "bv": f32(bv), "bo": f32(bo),
        "g1": f32(g1), "b1": f32(b1), "g2": f32(g2), "b2": f32(b2),
    }
    X = np.asarray(X, dtype=np.float32)
    Y = np.asarray(Y, dtype=np.float32)
    mask = np.asarray(mask)
    in_maps = []
    for b in range(8):
        mb = np.where(mask[b], np.float32(-1e4), np.float32(0.0)).astype(np.float32)
        in_maps.append({
            "XT": np.ascontiguousarray(X[b].T.astype(bf16)),
            "YT": np.ascontiguousarray(Y[b].T.astype(kvdt)),
            "MB": mb,
            **shared,
        })
    return in_maps


def kernel(X, Y, mask, Wq, bq, Wk, bk, Wv, bv, Wo, bo, g1, b1, g2, b2,
           _trace=False):
    nc = _get_nc()
    in_maps = make_in_maps(X, Y, mask, Wq, bq, Wk, bk, Wv, bv, Wo, bo,
                           g1, b1, g2, b2)
    res = run_bass_kernel_spmd(nc, in_maps, core_ids=list(range(8)),
                               trace=_trace)
    out = np.stack([np.ascontiguousarray(
        np.asarray(res.results[b]["OT"]).astype(np.float32).T)
        for b in range(8)])
    if _trace:
        return out, res
    return out
